# revision 1
# baseline (speedup 1.0000x reference)
"""Multi-head attention TRN2 kernel (b=4, n=2048, e=768, h=8 heads, d=96).

Sharding: 8 cores = 4 batches x 2 head-groups (4 heads each).
Each core computes, for its (batch, head-group):
    qkv projection (its heads' columns of Wqkv), per-head attention
    (softmax over full n=2048), and a partial output projection
    (its heads' rows of Wproj). Host sums the two partial outputs per
    batch (row-parallel linear unshard) and concatenates batches.

All matmul operands are float32r (full-rate PE, ~1e-4 relative rounding);
PSUM accumulation is fp32. Scores are computed transposed (ET[nk, nq]) so
no on-chip transposes are needed; softmax denominators come from an extra
ones-column appended to V (row 96 of the PV accumulator). exp() skips the
usual max-subtraction: logits/sqrt(e) for this problem are bounded (~|2|),
far from fp32 overflow. Per-head outputs are staged to DRAM in a
[row-block, 384, 128] layout so the output projection runs K=128-packed
(3 matmuls per chunk) with one contiguous reload per row block.
"""

import os

import numpy as np

import concourse.bacc as bacc
import concourse.mybir as mybir
import concourse.tile as tile
from concourse.bass_utils import run_bass_kernel_spmd

B, N, E = 4, 2048, 768
H = 8          # total heads
HL = 4         # heads per core
D = E // H     # 96
DH = D + 1     # 97 (with denominator column)
KB = E // 128  # 6 contraction blocks
NB = N // 128  # 16 row blocks
NC = 8         # cores
EL = HL * D    # 384 local e-dim
SCALE = float(E) ** -0.5

F32 = mybir.dt.float32
F32R = mybir.dt.float32r
AF = mybir.ActivationFunctionType
MULT = mybir.AluOpType.mult
ADD = mybir.AluOpType.add

_COMPILED = None
LAST_EXEC_NS = None
LAST_RESULTS = None


def _device_reset():
    """Recover a wedged NeuronCore (NRT_EXEC_UNIT_UNRECOVERABLE) via axon."""
    try:
        import ctypes
        import time

        import jax

        jax.devices()
        lib = ctypes.CDLL("/opt/axon/libaxon_pjrt.so")
        lib.axon_reset.restype = ctypes.c_int64
        lib.axon_reset()
        time.sleep(3)
    except Exception:
        pass


def _build():
    nc = bacc.Bacc("TRN2", target_bir_lowering=False, debug=False)

    xT_d = nc.dram_tensor("xT", [E, N], F32, kind="ExternalInput")
    wq_d = nc.dram_tensor("wq", [E, EL], F32, kind="ExternalInput")
    wk_d = nc.dram_tensor("wk", [E, EL], F32, kind="ExternalInput")
    wv_d = nc.dram_tensor("wv", [E, HL * DH], F32, kind="ExternalInput")
    bq_d = nc.dram_tensor("bq", [D, HL], F32, kind="ExternalInput")
    bk_d = nc.dram_tensor("bk", [D, HL], F32, kind="ExternalInput")
    bv_d = nc.dram_tensor("bv", [1, HL * DH], F32, kind="ExternalInput")
    wp_d = nc.dram_tensor("wp", [EL, E], F32, kind="ExternalInput")
    bp_d = nc.dram_tensor("bp", [1, E], F32, kind="ExternalInput")
    ones_d = nc.dram_tensor("ones", [1, 128], F32, kind="ExternalInput")
    out_d = nc.dram_tensor("out", [N, E], F32, kind="ExternalOutput")

    # attention outputs staged per output row-block, heads stacked along
    # partitions: ot_d[nb, h*96+dd, i] = OT_h[dd, nb*128+i]
    ot_d = nc.dram_tensor("ot_stage", [NB, EL, 128], F32R)

    with tile.TileContext(nc) as tc:
        with (
            tc.tile_pool(name="const", bufs=1) as cpool,
            tc.tile_pool(name="xt", bufs=1) as xpool,
            tc.tile_pool(name="qk", bufs=2) as qkpool,
            tc.tile_pool(name="vh", bufs=1) as vpool,
            tc.tile_pool(name="pt", bufs=3) as ptpool,
            tc.tile_pool(name="nrm", bufs=2) as npool,
            tc.tile_pool(name="pp", bufs=2, space="PSUM") as pp,
            tc.tile_pool(name="pattn", bufs=1, space="PSUM") as pattn,
        ):
            # ---- constants (DMA order matters: vproj prereqs first) ----
            wv_sb = []
            for kb in range(KB):
                t = cpool.tile([128, HL * DH], F32R, tag=f"wv{kb}")
                nc.gpsimd.dma_start(t[:], wv_d[kb * 128:(kb + 1) * 128, :])
                wv_sb.append(t)
            bv_sb = cpool.tile([1, HL * DH], F32R, tag="bv")
            nc.gpsimd.dma_start(bv_sb[:], bv_d[:])
            ones_sb = cpool.tile([1, 128], F32R, tag="ones")
            nc.gpsimd.dma_start(ones_sb[:], ones_d[:])

            # xT loads, chunked by column so downstream matmuls start early
            xT_sb = []
            for kb in range(KB):
                t = xpool.tile([128, N], F32R, tag=f"xt{kb}")
                xT_sb.append(t)
            for c in range(4):
                for kb in range(KB):
                    nc.gpsimd.dma_start(
                        xT_sb[kb][:, c * 512:(c + 1) * 512],
                        xT_d[kb * 128:(kb + 1) * 128, c * 512:(c + 1) * 512],
                    )
            wq_sb = []
            wk_sb = []
            for kb in range(KB):
                t = cpool.tile([128, EL], F32R, tag=f"wq{kb}")
                nc.gpsimd.dma_start(t[:], wq_d[kb * 128:(kb + 1) * 128, :])
                wq_sb.append(t)
                t = cpool.tile([128, EL], F32R, tag=f"wk{kb}")
                nc.gpsimd.dma_start(t[:], wk_d[kb * 128:(kb + 1) * 128, :])
                wk_sb.append(t)
            wp_sb = []
            for g in range(3):
                t = cpool.tile([128, E], F32R, tag=f"wp{g}")
                nc.gpsimd.dma_start(t[:], wp_d[g * 128:(g + 1) * 128, :])
                wp_sb.append(t)
            bp_sb = cpool.tile([1, E], F32R, tag="bp")
            nc.gpsimd.dma_start(bp_sb[:], bp_d[:])
            bq_sb = cpool.tile([D, HL], F32, tag="bq")
            nc.sync.dma_start(bq_sb[:], bq_d[:])
            bk_sb = cpool.tile([D, HL], F32, tag="bk")
            nc.sync.dma_start(bk_sb[:], bk_d[:])

            # broadcast bias tiles (one K=1 matmul each, reused everywhere)
            bvb_sb = cpool.tile([128, HL * DH], F32, tag="bvb")
            ps = pp.tile([128, 512], F32, tag="pp")
            nc.tensor.matmul(ps[:, 0:HL * DH], ones_sb[:], bv_sb[:], start=True, stop=True)
            nc.vector.tensor_copy(bvb_sb[:], ps[:, 0:HL * DH])
            bpb_sb = cpool.tile([128, E], F32, tag="bpb")

            # ---- V-hat projection: vhat[nb] [128, HL*97] (V + denom column) ----
            vhat = []
            with nc.named_scope("vproj"):
                for nb in range(NB):
                    ps = pp.tile([128, 512], F32, tag="pp")
                    for kb in range(KB):
                        nc.tensor.matmul(
                            ps[:, 0:HL * DH],
                            xT_sb[kb][:, nb * 128:(nb + 1) * 128],
                            wv_sb[kb][:],
                            start=(kb == 0),
                            stop=(kb == KB - 1),
                        )
                    vt = vpool.tile([128, HL * DH], F32R, tag=f"vh{nb}")
                    nc.vector.tensor_tensor(vt[:], ps[:, 0:HL * DH], bvb_sb[:], ADD)
                    vhat.append(vt)

            # ---- per-head: project qT/kT, attention; norm deferred one slot ----
            def emit_norm(job, after=(None, None)):
                h, qh, acc_sb = job
                with nc.named_scope(f"norm{h}_{qh}"):
                    sums = npool.tile([1, 1024], F32, tag="sums", bufs=1)
                    nc.vector.tensor_copy(sums[:], acc_sb[D:DH, :])
                    rec32 = npool.tile([1, 1024], F32, tag="rec32", bufs=1)
                    nc.vector.reciprocal_approx_fast(rec32[:], sums[:])
                    rec = npool.tile([1, 1024], F32R, tag="rec", bufs=1)
                    nc.vector.tensor_copy(rec[:], rec32[:])
                    for j in range(2):
                        c = 2 * qh + j
                        bc = pp.tile([128, 512], F32, tag="pp")
                        nc.tensor.matmul(
                            bc[0:D, :],
                            ones_sb[:, 0:D],
                            rec[:, j * 512:(j + 1) * 512],
                            start=True,
                            stop=True,
                        )
                        ot = npool.tile([D, 512], F32R, tag="ot")
                        nc.vector.tensor_tensor(
                            ot[:], acc_sb[0:D, j * 512:(j + 1) * 512], bc[0:D, :], MULT
                        )
                        # scatter the 4 row-blocks (off the sync queue)
                        for q in range(4):
                            nc.scalar.dma_start(
                                ot_d[c * 4 + q, h * D:(h + 1) * D, :],
                                ot[:, q * 128:(q + 1) * 128],
                            )
                        if after[j] is not None:
                            after[j]()

            def start_qkproj(h):
                with nc.named_scope(f"qkproj{h}"):
                    qT = qkpool.tile([D, N], F32R, tag="qT", name=f"qT{h}")
                    kT = qkpool.tile([D, N], F32R, tag="kT", name=f"kT{h}")
                return (qT, kT)

            def emit_qkproj_chunk(h, tiles, i):
                qT, kT = tiles
                qk, c = divmod(i, 4)
                w_sb, b_sb, dst, sc = [
                    (wq_sb, bq_sb, qT, SCALE),
                    (wk_sb, bk_sb, kT, 1.0),
                ][qk]
                with nc.named_scope(f"qkproj{h}"):
                    ps = pp.tile([128, 512], F32, tag="pp", name=f"psqk{h}_{i}")
                    for kb in range(KB):
                        nc.tensor.matmul(
                            ps[0:D, :],
                            w_sb[kb][:, h * D:(h + 1) * D],
                            xT_sb[kb][:, c * 512:(c + 1) * 512],
                            start=(kb == 0),
                            stop=(kb == KB - 1),
                        )
                    nc.vector.tensor_scalar(
                        dst[:, c * 512:(c + 1) * 512],
                        ps[0:D, :],
                        sc,
                        b_sb[:, h:h + 1],
                        MULT,
                        ADD,
                    )

            def emit_out(nb):
                otn = npool.tile([128, 3, 128], F32R, tag="otn", bufs=4)
                src = ot_d[nb].rearrange("(g p) i -> p g i", p=128)
                eng = nc.sync if nb % 2 == 0 else nc.scalar
                eng.dma_start(otn[:], src)
                po = pattn.tile([128, E], F32, tag="et", bufs=2)
                for off, w in [(0, 512), (512, 256)]:
                    for g in range(3):
                        nc.tensor.matmul(
                            po[:, off:off + w],
                            otn[:, g, :],
                            wp_sb[g][:, off:off + w],
                            start=(g == 0),
                            stop=(g == 2),
                        )
                osb = npool.tile([128, E], F32, tag="osb", bufs=3)
                nc.vector.tensor_tensor(osb[:], po[:], bpb_sb[:], ADD)
                nc.gpsimd.dma_start(out_d[nb * 128:(nb + 1) * 128, :], osb[:])

            pending = None
            tiles = start_qkproj(0)
            for i in range(8):
                emit_qkproj_chunk(0, tiles, i)
            next_tiles = None
            for h in range(HL):
                qT, kT = tiles
                for qh in range(2):
                    with nc.named_scope(f"attn{h}_{qh}"):
                        acc = pattn.tile([DH, 1024], F32, tag="acc")

                        def emit_pv(kbp, pt):
                            for j in range(2):
                                nc.tensor.matmul(
                                    acc[:, j * 512:(j + 1) * 512],
                                    vhat[kbp][:, h * DH:(h + 1) * DH],
                                    pt[:, j * 512:(j + 1) * 512],
                                    start=(kbp == 0),
                                    stop=(kbp == NB - 1),
                                )

                        prev = None
                        for kb in range(NB):
                            et = pattn.tile([128, 1024], F32, tag="et", bufs=2)
                            for j in range(2):
                                c = 2 * qh + j
                                nc.tensor.matmul(
                                    et[:, j * 512:(j + 1) * 512],
                                    kT[:, kb * 128:(kb + 1) * 128],
                                    qT[:, c * 512:(c + 1) * 512],
                                    start=True,
                                    stop=True,
                                )
                            # PV runs one step behind so exp(kb) overlaps it
                            if prev is not None:
                                emit_pv(kb - 1, prev)
                            pt = ptpool.tile([128, 1024], F32R, tag="pt")
                            nc.scalar.activation(pt[:], et[:], AF.Exp)
                            prev = pt
                            if kb == 6 and pending is not None:
                                emit_norm(pending)
                                pending = None
                            if qh == 1 and h + 1 < HL:
                                # interleave next head's projections into the
                                # exp-wait gaps of this attention pass
                                if kb == 0:
                                    next_tiles = start_qkproj(h + 1)
                                if kb % 2 == 1:
                                    emit_qkproj_chunk(h + 1, next_tiles, kb // 2)
                        emit_pv(NB - 1, prev)
                        acc_sb = npool.tile([DH, 1024], F32, tag="acc_sb")
                        nc.vector.tensor_copy(acc_sb[:, 0:512], acc[:, 0:512])
                        nc.scalar.copy(acc_sb[:, 512:1024], acc[:, 512:1024])
                        pending = (h, qh, acc_sb)
                tiles = next_tiles
                if h == 0:
                    # build the bproj broadcast late (off the critical start)
                    for off, w in [(0, 512), (512, 256)]:
                        ps = pp.tile([128, 512], F32, tag="pp")
                        nc.tensor.matmul(
                            ps[:, 0:w], ones_sb[:], bp_sb[:, off:off + w],
                            start=True, stop=True,
                        )
                        nc.vector.tensor_copy(bpb_sb[:, off:off + w], ps[:, 0:w])

            # ---- output projection out[n, e] = OT^T @ Wp + bp ----
            # nb 0..7 only needs qh=0 data; run them before the last norm's
            # (h3, qh=1) chain so that chain overlaps PE work.

            for nb in range(8):
                emit_out(nb)
            emit_norm(
                pending,
                after=(
                    lambda: [emit_out(nb) for nb in range(8, 12)],
                    lambda: [emit_out(nb) for nb in range(12, NB)],
                ),
            )

    nc.compile()
    return nc


def _shard(x, Wqkv, bqkv, Wproj, bproj):
    """Build per-core input maps. Core c -> (batch c//2, head-group c%2)."""
    Wr = np.ascontiguousarray(Wqkv.reshape(E, H, D, 3))
    br = np.ascontiguousarray(bqkv.reshape(H, D, 3))
    ones = np.ones((1, 128), np.float32)
    in_maps = []
    for c in range(NC):
        bb, hg = divmod(c, 2)
        hs = slice(hg * HL, (hg + 1) * HL)
        wq = np.ascontiguousarray(Wr[:, hs, :, 0].reshape(E, EL))
        wk = np.ascontiguousarray(Wr[:, hs, :, 1].reshape(E, EL))
        wv = np.zeros((E, HL, DH), np.float32)
        wv[:, :, :D] = Wr[:, hs, :, 2]
        bq = np.ascontiguousarray((br[hs, :, 0] * SCALE).T)  # [D, HL], pre-scaled
        bk = np.ascontiguousarray(br[hs, :, 1].T)
        bv = np.zeros((HL, DH), np.float32)
        bv[:, :D] = br[hs, :, 2]
        bv[:, D] = 1.0  # denominator ones column
        wp = np.ascontiguousarray(Wproj[hg * EL:(hg + 1) * EL, :])
        bp = bproj if hg == 0 else np.zeros_like(bproj)
        in_maps.append({
            "xT": np.ascontiguousarray(x[bb].T),
            "wq": wq,
            "wk": wk,
            "wv": np.ascontiguousarray(wv.reshape(E, HL * DH)),
            "bq": bq,
            "bk": bk,
            "bv": np.ascontiguousarray(bv.reshape(1, HL * DH)),
            "wp": wp,
            "bp": np.ascontiguousarray(bp.reshape(1, E)),
            "ones": ones,
        })
    return in_maps


def kernel(x, Wqkv, bqkv, Wproj, bproj):
    global _COMPILED, LAST_EXEC_NS, LAST_RESULTS
    x = np.asarray(x, dtype=np.float32)
    Wqkv = np.asarray(Wqkv, dtype=np.float32)
    bqkv = np.asarray(bqkv, dtype=np.float32)
    Wproj = np.asarray(Wproj, dtype=np.float32)
    bproj = np.asarray(bproj, dtype=np.float32)

    if _COMPILED is None:
        _COMPILED = _build()
    nc = _COMPILED

    in_maps = _shard(x, Wqkv, bqkv, Wproj, bproj)
    trace = bool(int(os.environ.get("BASS_MHA_TRACE", "0")))
    try:
        res = run_bass_kernel_spmd(nc, in_maps, list(range(NC)), trace=trace)
    except Exception:
        _device_reset()
        res = run_bass_kernel_spmd(nc, in_maps, list(range(NC)), trace=trace)
    LAST_EXEC_NS = res.exec_time_ns
    LAST_RESULTS = res

    out = np.empty((B, N, E), np.float32)
    for bb in range(B):
        out[bb] = res.results[2 * bb]["out"] + res.results[2 * bb + 1]["out"]
    return out



# revision 5
# speedup vs baseline: 1.0622x; 1.0622x over previous
"""Multi-head attention TRN2 kernel (b=4, n=2048, e=768, h=8 heads, d=96).

Sharding: 8 cores = 4 batches x 2 head-groups (4 heads each).
Each core computes, for its (batch, head-group):
    qkv projection (its heads' columns of Wqkv), per-head attention
    (softmax over full n=2048), and a partial output projection
    (its heads' rows of Wproj). Host sums the two partial outputs per
    batch (row-parallel linear unshard) and concatenates batches.

All matmul operands are bf16 (PE runs 1 col/cycle for both bf16 and
f32r, but bf16 halves SBUF footprint and input DMA, letting all four
heads' qT/kT stay resident); PSUM accumulation is fp32. Scores are
computed transposed (ET[nk, nq]) so no on-chip transposes are needed;
softmax denominators come from an extra ones-column appended to V
(row 96 of the PV accumulator). exp() skips max-subtraction: logits
are bounded (~|2|) for this problem.

Passes run qh-major ((h0..h3, qh=0) then (h0..h3, qh=1)) so that by the
q1 passes all qh=0 norms are done and the output projection for the
first half of the rows interleaves into the exp-gated bubbles of the
remaining attention passes. Each pass's kb loop is fed a paced queue of
independent PE "fill jobs" (next head's QK projection, deferred
q1-column projections, V projection tail, out-proj blocks) so the PE
never waits on the scalar-engine exp chain.
"""

import os

import ml_dtypes
import numpy as np

import concourse.bacc as bacc
import concourse.mybir as mybir
import concourse.tile as tile
from concourse.bass_utils import run_bass_kernel_spmd

B, N, E = 4, 2048, 768
H = 8          # total heads
HL = 4         # heads per core
D = E // H     # 96
DH = D + 1     # 97 (with denominator column)
KB = E // 128  # 6 contraction blocks
NB = N // 128  # 16 row blocks
NC = 8         # cores
EL = HL * D    # 384 local e-dim
SCALE = float(E) ** -0.5

F32 = mybir.dt.float32
F32R = mybir.dt.float32r
BF16 = mybir.dt.bfloat16
AF = mybir.ActivationFunctionType
MULT = mybir.AluOpType.mult
ADD = mybir.AluOpType.add

_COMPILED = None
LAST_EXEC_NS = None
LAST_RESULTS = None


def _device_reset():
    """Recover a wedged NeuronCore (NRT_EXEC_UNIT_UNRECOVERABLE) via axon."""
    try:
        import ctypes
        import time

        import jax

        jax.devices()
        lib = ctypes.CDLL("/opt/axon/libaxon_pjrt.so")
        lib.axon_reset.restype = ctypes.c_int64
        lib.axon_reset()
        time.sleep(3)
    except Exception:
        pass


def _build():
    nc = bacc.Bacc("TRN2", target_bir_lowering=False, debug=False)

    xT_d = nc.dram_tensor("xT", [E, N], BF16, kind="ExternalInput")
    wq_d = nc.dram_tensor("wq", [E, EL], BF16, kind="ExternalInput")
    wk_d = nc.dram_tensor("wk", [E, EL], BF16, kind="ExternalInput")
    wv_d = nc.dram_tensor("wv", [E, HL * DH], BF16, kind="ExternalInput")
    bq_d = nc.dram_tensor("bq", [D, HL], F32, kind="ExternalInput")
    bk_d = nc.dram_tensor("bk", [D, HL], F32, kind="ExternalInput")
    bv_d = nc.dram_tensor("bv", [1, HL * DH], F32, kind="ExternalInput")
    wp_d = nc.dram_tensor("wp", [EL, E], BF16, kind="ExternalInput")
    bp_d = nc.dram_tensor("bp", [1, E], F32, kind="ExternalInput")
    ones_d = nc.dram_tensor("ones", [1, 128], F32, kind="ExternalInput")
    out_d = nc.dram_tensor("out", [N, E], F32, kind="ExternalOutput")

    # attention outputs staged per output row-block, heads stacked along
    # partitions: ot_d[nb, h*96+dd, i] = OT_h[dd, nb*128+i]
    ot_d = nc.dram_tensor("ot_stage", [NB, EL, 128], BF16)

    with tile.TileContext(nc) as tc:
        with (
            tc.tile_pool(name="const", bufs=1) as cpool,
            tc.tile_pool(name="xt", bufs=1) as xpool,
            tc.tile_pool(name="vh", bufs=1) as vpool,
            tc.tile_pool(name="pt", bufs=3) as ptpool,
            tc.tile_pool(name="nrm", bufs=2) as npool,
            tc.tile_pool(name="pp", bufs=2, space="PSUM") as pp,
            tc.tile_pool(name="pattn", bufs=1, space="PSUM") as pattn,
        ):
            # ---- input DMA (queue split: gpsimd=xT, sync=w, scalar=small) ----
            wq_sb = []
            wk_sb = []
            for kb in range(KB):
                t = cpool.tile([128, EL], BF16, tag=f"wq{kb}")
                nc.sync.dma_start(t[:], wq_d[kb * 128:(kb + 1) * 128, :])
                wq_sb.append(t)
                t = cpool.tile([128, EL], BF16, tag=f"wk{kb}")
                nc.sync.dma_start(t[:], wk_d[kb * 128:(kb + 1) * 128, :])
                wk_sb.append(t)
            xT_sb = []
            for kb in range(KB):
                t = xpool.tile([128, N], BF16, tag=f"xt{kb}")
                xT_sb.append(t)
            for c in range(4):
                for kb in range(KB):
                    nc.gpsimd.dma_start(
                        xT_sb[kb][:, c * 512:(c + 1) * 512],
                        xT_d[kb * 128:(kb + 1) * 128, c * 512:(c + 1) * 512],
                    )
            wv_sb = []
            for kb in range(KB):
                t = cpool.tile([128, HL * DH], BF16, tag=f"wv{kb}")
                nc.scalar.dma_start(t[:], wv_d[kb * 128:(kb + 1) * 128, :])
                wv_sb.append(t)
            bv_sb = cpool.tile([1, HL * DH], F32R, tag="bv")
            nc.gpsimd.dma_start(bv_sb[:], bv_d[:])
            ones_sb = cpool.tile([1, 128], F32R, tag="ones")
            nc.gpsimd.dma_start(ones_sb[:], ones_d[:])
            bq_sb = cpool.tile([D, HL], F32, tag="bq")
            nc.scalar.dma_start(bq_sb[:], bq_d[:])
            bk_sb = cpool.tile([D, HL], F32, tag="bk")
            nc.scalar.dma_start(bk_sb[:], bk_d[:])
            wp_sb = []
            for g in range(3):
                t = cpool.tile([128, E], BF16, tag=f"wp{g}")
                nc.sync.dma_start(t[:], wp_d[g * 128:(g + 1) * 128, :])
                wp_sb.append(t)
            bp_sb = cpool.tile([1, E], F32R, tag="bp")
            nc.gpsimd.dma_start(bp_sb[:], bp_d[:])

            # broadcast V bias (one K=1 matmul, reused by every vproj block)
            bvb_sb = cpool.tile([128, HL * DH], F32, tag="bvb")
            ps = pp.tile([128, 512], F32, tag="pp")
            nc.tensor.matmul(ps[:, 0:HL * DH], ones_sb[:], bv_sb[:], start=True, stop=True)
            nc.vector.tensor_copy(bvb_sb[:], ps[:, 0:HL * DH])
            bpb_sb = cpool.tile([128, E], F32, tag="bpb")

            # persistent per-head qT/kT (bf16 halves SBUF: all 8 stay live)
            qT = [
                cpool.tile([D, N], BF16, tag=f"qT{h}", name=f"qT{h}")
                for h in range(HL)
            ]
            kT = [
                cpool.tile([D, N], BF16, tag=f"kT{h}", name=f"kT{h}")
                for h in range(HL)
            ]
            vhat = [
                vpool.tile([128, HL * DH], BF16, tag=f"vh{nb}", name=f"vh{nb}")
                for nb in range(NB)
            ]

            def emit_vproj(nb):
                with nc.named_scope(f"vproj{nb}"):
                    ps = pp.tile([128, 512], F32, tag="pp")
                    for kb in range(KB):
                        nc.tensor.matmul(
                            ps[:, 0:HL * DH],
                            xT_sb[kb][:, nb * 128:(nb + 1) * 128],
                            wv_sb[kb][:],
                            start=(kb == 0),
                            stop=(kb == KB - 1),
                        )
                    nc.vector.tensor_tensor(vhat[nb][:], ps[:, 0:HL * DH], bvb_sb[:], ADD)

            def emit_qk_chunk(h, i):
                """i in 0..7 -> (q|k, column chunk c)."""
                qk, c = divmod(i, 4)
                w_sb, b_sb, dst, sc = [
                    (wq_sb, bq_sb, qT[h], SCALE),
                    (wk_sb, bk_sb, kT[h], 1.0),
                ][qk]
                with nc.named_scope(f"qkproj{h}"):
                    ps = pp.tile([128, 512], F32, tag="pp", name=f"psqk{h}_{i}")
                    for kb in range(KB):
                        nc.tensor.matmul(
                            ps[0:D, :],
                            w_sb[kb][:, h * D:(h + 1) * D],
                            xT_sb[kb][:, c * 512:(c + 1) * 512],
                            start=(kb == 0),
                            stop=(kb == KB - 1),
                        )
                    nc.vector.tensor_scalar(
                        dst[:, c * 512:(c + 1) * 512],
                        ps[0:D, :],
                        sc,
                        b_sb[:, h:h + 1],
                        MULT,
                        ADD,
                    )

            def emit_bpb():
                with nc.named_scope("bpb"):
                    for off, w in [(0, 512), (512, 256)]:
                        ps = pp.tile([128, 512], F32, tag="pp")
                        nc.tensor.matmul(
                            ps[:, 0:w], ones_sb[:], bp_sb[:, off:off + w],
                            start=True, stop=True,
                        )
                        nc.vector.tensor_copy(bpb_sb[:, off:off + w], ps[:, 0:w])

            def emit_norm(job, after=(None, None)):
                h, qh, acc_sb = job
                with nc.named_scope(f"norm{h}_{qh}"):
                    sums = npool.tile([1, 1024], F32, tag="sums", bufs=1)
                    nc.vector.tensor_copy(sums[:], acc_sb[D:DH, :])
                    rec32 = npool.tile([1, 1024], F32, tag="rec32", bufs=1)
                    nc.vector.reciprocal_approx_fast(rec32[:], sums[:])
                    rec = npool.tile([1, 1024], F32R, tag="rec", bufs=1)
                    nc.vector.tensor_copy(rec[:], rec32[:])
                    for j in range(2):
                        c = 2 * qh + j
                        bc = pp.tile([128, 512], F32, tag="pp")
                        nc.tensor.matmul(
                            bc[0:D, :],
                            ones_sb[:, 0:D],
                            rec[:, j * 512:(j + 1) * 512],
                            start=True,
                            stop=True,
                        )
                        ot = npool.tile([D, 512], BF16, tag="ot")
                        nc.vector.tensor_tensor(
                            ot[:], acc_sb[0:D, j * 512:(j + 1) * 512], bc[0:D, :], MULT
                        )
                        # scatter the 4 row-blocks (sync queue: idle by now)
                        for q in range(4):
                            nc.sync.dma_start(
                                ot_d[c * 4 + q, h * D:(h + 1) * D, :],
                                ot[:, q * 128:(q + 1) * 128],
                            )
                        if after[j] is not None:
                            after[j]()

            def emit_out(nb):
                with nc.named_scope(f"oproj{nb}"):
                    otn = npool.tile([128, 3, 128], BF16, tag="otn", bufs=4)
                    src = ot_d[nb].rearrange("(g p) i -> p g i", p=128)
                    eng = nc.gpsimd if nb % 2 == 0 else nc.scalar
                    eng.dma_start(otn[:], src)
                    osb = npool.tile([128, E], F32, tag="osb", bufs=3)
                    for off, w in [(0, 512), (512, 256)]:
                        po = pp.tile([128, 512], F32, tag="pp")
                        for g in range(3):
                            nc.tensor.matmul(
                                po[:, 0:w],
                                otn[:, g, :],
                                wp_sb[g][:, off:off + w],
                                start=(g == 0),
                                stop=(g == 2),
                            )
                        nc.vector.tensor_tensor(
                            osb[:, off:off + w], po[:, 0:w], bpb_sb[:, off:off + w], ADD
                        )
                    nc.gpsimd.dma_start(out_d[nb * 128:(nb + 1) * 128, :], osb[:])

            # ---- pre-phase: h0 critical qk chunks, then V projection ----
            # critical = q chunks for qh0 (c0,c1) + all k chunks; q c2/c3 are
            # only needed by the qh=1 pass and are deferred as fill jobs.
            CRIT = [0, 4, 1, 5, 6, 7]   # (q,c0),(k,c0),(q,c1),(k,c1),(k,c2),(k,c3)
            DEFER = [2, 3]              # (q,c2),(q,c3)
            for i in CRIT:
                emit_qk_chunk(0, i)
            for nb in range(12):
                emit_vproj(nb)

            # fill-job queues per pass: (min_kb, closure)
            def J(fn, *a, min_kb=0):
                return (min_kb, lambda: fn(*a))

            fills = {
                (0, 0): [J(emit_vproj, nb) for nb in range(12, NB)]
                        + [J(emit_qk_chunk, 1, i) for i in CRIT],
                (1, 0): [J(emit_qk_chunk, 2, i) for i in CRIT]
                        + [J(emit_qk_chunk, 0, i) for i in DEFER],
                (2, 0): [J(emit_qk_chunk, 3, i) for i in CRIT]
                        + [J(emit_qk_chunk, 1, i) for i in DEFER],
                (3, 0): [J(emit_qk_chunk, 2, i) for i in DEFER]
                        + [J(emit_qk_chunk, 3, i) for i in DEFER]
                        + [J(emit_bpb)],
                # q1 passes: out-proj for the qh=0 half interleaves in.
                # nb0-7 need every head's qh=0 norm; the last one (h3,q0) is
                # emitted at kb==6 of pass (0,1), so gate on kb>=8 there.
                (0, 1): [J(emit_out, nb, min_kb=8) for nb in range(0, 2)],
                (1, 1): [J(emit_out, nb) for nb in range(2, 5)],
                (2, 1): [J(emit_out, nb) for nb in range(5, 8)],
                (3, 1): [],
            }

            pending = None
            for qh in range(2):
                for h in range(HL):
                    jobs = fills[(h, qh)]
                    popped = 0
                    with nc.named_scope(f"attn{h}_{qh}"):
                        acc = pattn.tile([DH, 1024], F32, tag="acc")

                        def emit_pv(kbp, pt):
                            for j in range(2):
                                nc.tensor.matmul(
                                    acc[:, j * 512:(j + 1) * 512],
                                    vhat[kbp][:, h * DH:(h + 1) * DH],
                                    pt[:, j * 512:(j + 1) * 512],
                                    start=(kbp == 0),
                                    stop=(kbp == NB - 1),
                                )

                        prev = None
                        for kb in range(NB):
                            et = pattn.tile([128, 1024], F32, tag="et", bufs=2)
                            for j in range(2):
                                c = 2 * qh + j
                                nc.tensor.matmul(
                                    et[:, j * 512:(j + 1) * 512],
                                    kT[h][:, kb * 128:(kb + 1) * 128],
                                    qT[h][:, c * 512:(c + 1) * 512],
                                    start=True,
                                    stop=True,
                                )
                            # PV runs one step behind so exp(kb) overlaps it
                            if prev is not None:
                                emit_pv(kb - 1, prev)
                            pt = ptpool.tile([128, 1024], BF16, tag="pt")
                            nc.scalar.activation(pt[:], et[:], AF.Exp)
                            prev = pt
                            if kb == 6 and pending is not None:
                                emit_norm(pending)
                                pending = None
                            # paced fill: spread jobs evenly over eligible kbs
                            target = (kb + 1) * len(jobs) // NB
                            while popped < len(jobs) and popped < target \
                                    and jobs[popped][0] <= kb:
                                jobs[popped][1]()
                                popped += 1
                        while popped < len(jobs):
                            jobs[popped][1]()
                            popped += 1
                        emit_pv(NB - 1, prev)
                        acc_sb = npool.tile([DH, 1024], F32, tag="acc_sb")
                        nc.vector.tensor_copy(acc_sb[:, 0:512], acc[:, 0:512])
                        nc.scalar.copy(acc_sb[:, 512:1024], acc[:, 512:1024])
                        pending = (h, qh, acc_sb)

            # ---- tail: last norm overlapped with remaining out-proj ----
            emit_norm(
                pending,
                after=(
                    lambda: [emit_out(nb) for nb in range(8, 12)],
                    lambda: [emit_out(nb) for nb in range(12, NB)],
                ),
            )

    nc.compile()
    return nc


def _shard(x, Wqkv, bqkv, Wproj, bproj):
    """Build per-core input maps. Core c -> (batch c//2, head-group c%2)."""
    BF = ml_dtypes.bfloat16
    Wr = np.ascontiguousarray(Wqkv.reshape(E, H, D, 3))
    br = np.ascontiguousarray(bqkv.reshape(H, D, 3))
    ones = np.ones((1, 128), np.float32)
    in_maps = []
    for c in range(NC):
        bb, hg = divmod(c, 2)
        hs = slice(hg * HL, (hg + 1) * HL)
        wq = np.ascontiguousarray(Wr[:, hs, :, 0].reshape(E, EL).astype(BF))
        wk = np.ascontiguousarray(Wr[:, hs, :, 1].reshape(E, EL).astype(BF))
        wv = np.zeros((E, HL, DH), np.float32)
        wv[:, :, :D] = Wr[:, hs, :, 2]
        bq = np.ascontiguousarray((br[hs, :, 0] * SCALE).T)  # [D, HL], pre-scaled
        bk = np.ascontiguousarray(br[hs, :, 1].T)
        bv = np.zeros((HL, DH), np.float32)
        bv[:, :D] = br[hs, :, 2]
        bv[:, D] = 1.0  # denominator ones column
        wp = np.ascontiguousarray(Wproj[hg * EL:(hg + 1) * EL, :].astype(BF))
        bp = bproj if hg == 0 else np.zeros_like(bproj)
        in_maps.append({
            "xT": np.ascontiguousarray(x[bb].T.astype(BF)),
            "wq": wq,
            "wk": wk,
            "wv": np.ascontiguousarray(wv.reshape(E, HL * DH).astype(BF)),
            "bq": bq,
            "bk": bk,
            "bv": np.ascontiguousarray(bv.reshape(1, HL * DH)),
            "wp": wp,
            "bp": np.ascontiguousarray(bp.reshape(1, E)),
            "ones": ones,
        })
    return in_maps


def kernel(x, Wqkv, bqkv, Wproj, bproj):
    global _COMPILED, LAST_EXEC_NS, LAST_RESULTS
    x = np.asarray(x, dtype=np.float32)
    Wqkv = np.asarray(Wqkv, dtype=np.float32)
    bqkv = np.asarray(bqkv, dtype=np.float32)
    Wproj = np.asarray(Wproj, dtype=np.float32)
    bproj = np.asarray(bproj, dtype=np.float32)

    if _COMPILED is None:
        _COMPILED = _build()
    nc = _COMPILED

    in_maps = _shard(x, Wqkv, bqkv, Wproj, bproj)
    trace = bool(int(os.environ.get("BASS_MHA_TRACE", "0")))
    try:
        res = run_bass_kernel_spmd(nc, in_maps, list(range(NC)), trace=trace)
    except Exception:
        _device_reset()
        res = run_bass_kernel_spmd(nc, in_maps, list(range(NC)), trace=trace)
    LAST_EXEC_NS = res.exec_time_ns
    LAST_RESULTS = res

    out = np.empty((B, N, E), np.float32)
    for bb in range(B):
        out[bb] = res.results[2 * bb]["out"] + res.results[2 * bb + 1]["out"]
    return out


# revision 12
# speedup vs baseline: 1.1085x; 1.0436x over previous
"""Multi-head attention TRN2 kernel (b=4, n=2048, e=768, h=8 heads, d=96).

Sharding: 8 cores = 4 batches x 2 head-groups (4 heads each).
Each core computes, for its (batch, head-group):
    qkv projection (its heads' columns of Wqkv), per-head attention
    (softmax over full n=2048), and a partial output projection
    (its heads' rows of Wproj). Host sums the two partial outputs per
    batch (row-parallel linear unshard) and concatenates batches.

All matmul operands are bf16 (PE runs 1 col/cycle for both bf16 and
f32r, but bf16 halves SBUF footprint and input DMA, letting all four
heads' qT/kT stay resident); PSUM accumulation is fp32. Scores are
computed transposed (ET[nk, nq]) so no on-chip transposes are needed;
softmax denominators come from an extra ones-column appended to V
(row 96 of the PV accumulator). exp() skips max-subtraction: logits
are bounded (~|2|) for this problem.

Passes run qh-major ((h0..h3, qh=0) then (h0..h3, qh=1)) so that by the
q1 passes all qh=0 norms are done and the output projection for the
first half of the rows interleaves into the exp-gated bubbles of the
remaining attention passes. Each pass's kb loop is fed a paced queue of
independent PE "fill jobs" (next head's QK projection, deferred
q1-column projections, V projection tail, out-proj blocks) so the PE
never waits on the scalar-engine exp chain.
"""

import os

import ml_dtypes
import numpy as np

import concourse.bacc as bacc
import concourse.mybir as mybir
import concourse.tile as tile
from concourse.bass_utils import run_bass_kernel_spmd

B, N, E = 4, 2048, 768
H = 8          # total heads
HL = 4         # heads per core
D = E // H     # 96
DH = D + 1     # 97 (with denominator column)
KB = E // 128  # 6 contraction blocks
NB = N // 128  # 16 row blocks
NC = 8         # cores
EL = HL * D    # 384 local e-dim
SCALE = float(E) ** -0.5

F32 = mybir.dt.float32
F32R = mybir.dt.float32r
BF16 = mybir.dt.bfloat16
AF = mybir.ActivationFunctionType
MULT = mybir.AluOpType.mult
ADD = mybir.AluOpType.add

_COMPILED = None
LAST_EXEC_NS = None
LAST_RESULTS = None


def _device_reset():
    """Recover a wedged NeuronCore (NRT_EXEC_UNIT_UNRECOVERABLE) via axon."""
    try:
        import ctypes
        import time

        import jax

        jax.devices()
        lib = ctypes.CDLL("/opt/axon/libaxon_pjrt.so")
        lib.axon_reset.restype = ctypes.c_int64
        lib.axon_reset()
        time.sleep(3)
    except Exception:
        pass


def _build():
    nc = bacc.Bacc("TRN2", target_bir_lowering=False, debug=False)

    xT_d = nc.dram_tensor("xT", [E, N], BF16, kind="ExternalInput")
    wq_d = nc.dram_tensor("wq", [E, EL], BF16, kind="ExternalInput")
    wk_d = nc.dram_tensor("wk", [E, EL], BF16, kind="ExternalInput")
    wv_d = nc.dram_tensor("wv", [E, HL * DH], BF16, kind="ExternalInput")
    bq_d = nc.dram_tensor("bq", [D, HL], F32, kind="ExternalInput")
    bk_d = nc.dram_tensor("bk", [D, HL], F32, kind="ExternalInput")
    bv_d = nc.dram_tensor("bv", [1, HL * DH], F32, kind="ExternalInput")
    wp_d = nc.dram_tensor("wp", [EL, E], BF16, kind="ExternalInput")
    wph_d = nc.dram_tensor("wph", [HL, D, E], BF16, kind="ExternalInput")
    bp_d = nc.dram_tensor("bp", [1, E], F32, kind="ExternalInput")
    ones_d = nc.dram_tensor("ones", [1, 128], F32, kind="ExternalInput")
    out_d = nc.dram_tensor("out", [N, E], F32, kind="ExternalOutput")

    # qh=0 attention outputs staged per output row-block, heads stacked
    # along partitions: ot_d[nb, h*96+dd, i] = OT_h[dd, nb*128+i].
    # The qh=1 half skips staging: its normalized outputs stay in SBUF and
    # the tail projects them directly (4 per-head K=96 matmuls per block).
    ot_d = nc.dram_tensor("ot_stage", [NB // 2, EL, 128], BF16)

    with tile.TileContext(nc) as tc:
        with (
            tc.tile_pool(name="const", bufs=1) as cpool,
            tc.tile_pool(name="xt", bufs=1) as xpool,
            tc.tile_pool(name="vh", bufs=1) as vpool,
            tc.tile_pool(name="pt", bufs=3) as ptpool,
            tc.tile_pool(name="nrm", bufs=2) as npool,
            tc.tile_pool(name="pp", bufs=2, space="PSUM") as pp,
            tc.tile_pool(name="pattn", bufs=1, space="PSUM") as pattn,
        ):
            # ---- input DMA (queue split: gpsimd=xT, sync=w, scalar=small).
            # Order within each queue = need order: ones/bv lead gpsimd so
            # the bvb broadcast matmul (first PE op) isn't stuck behind xT.
            bv_sb = cpool.tile([1, HL * DH], F32R, tag="bv")
            nc.gpsimd.dma_start(bv_sb[:], bv_d[:])
            ones_sb = cpool.tile([1, 128], F32R, tag="ones")
            nc.gpsimd.dma_start(ones_sb[:], ones_d[:])
            wq_sb = []
            wk_sb = []
            for kb in range(KB):
                t = cpool.tile([128, EL], BF16, tag=f"wq{kb}")
                nc.sync.dma_start(t[:], wq_d[kb * 128:(kb + 1) * 128, :])
                wq_sb.append(t)
                t = cpool.tile([128, EL], BF16, tag=f"wk{kb}")
                nc.sync.dma_start(t[:], wk_d[kb * 128:(kb + 1) * 128, :])
                wk_sb.append(t)
            xT_sb = []
            for kb in range(KB):
                t = xpool.tile([128, N], BF16, tag=f"xt{kb}")
                xT_sb.append(t)
            for c in range(4):
                for kb in range(KB):
                    nc.gpsimd.dma_start(
                        xT_sb[kb][:, c * 512:(c + 1) * 512],
                        xT_d[kb * 128:(kb + 1) * 128, c * 512:(c + 1) * 512],
                    )
            bq_sb = cpool.tile([D, HL], F32, tag="bq")
            nc.scalar.dma_start(bq_sb[:], bq_d[:])
            bk_sb = cpool.tile([D, HL], F32, tag="bk")
            nc.scalar.dma_start(bk_sb[:], bk_d[:])
            wv_sb = []
            for kb in range(KB):
                t = cpool.tile([128, HL * DH], BF16, tag=f"wv{kb}")
                nc.scalar.dma_start(t[:], wv_d[kb * 128:(kb + 1) * 128, :])
                wv_sb.append(t)
            wp_sb = []
            for g in range(3):
                t = cpool.tile([128, E], BF16, tag=f"wp{g}")
                nc.sync.dma_start(t[:], wp_d[g * 128:(g + 1) * 128, :])
                wp_sb.append(t)
            wph_sb = []
            for h in range(HL):
                t = cpool.tile([D, E], BF16, tag=f"wph{h}")
                nc.sync.dma_start(t[:], wph_d[h])
                wph_sb.append(t)
            bp_sb = cpool.tile([1, E], F32R, tag="bp")
            nc.gpsimd.dma_start(bp_sb[:], bp_d[:])

            # broadcast V bias (one K=1 matmul, reused by every vproj block)
            bvb_sb = cpool.tile([128, HL * DH], F32, tag="bvb")
            ps = pp.tile([128, 512], F32, tag="pp")
            nc.tensor.matmul(ps[:, 0:HL * DH], ones_sb[:], bv_sb[:], start=True, stop=True)
            nc.vector.tensor_copy(bvb_sb[:], ps[:, 0:HL * DH])
            bpb_sb = cpool.tile([128, E], F32, tag="bpb")

            # persistent per-head qT/kT (bf16 halves SBUF: all 8 stay live)
            qT = [
                cpool.tile([D, N], BF16, tag=f"qT{h}", name=f"qT{h}")
                for h in range(HL)
            ]
            kT = [
                cpool.tile([D, N], BF16, tag=f"kT{h}", name=f"kT{h}")
                for h in range(HL)
            ]
            vhat = [
                vpool.tile([128, HL * DH], BF16, tag=f"vh{nb}", name=f"vh{nb}")
                for nb in range(NB)
            ]

            def emit_vproj(nb):
                with nc.named_scope(f"vproj{nb}"):
                    ps = pp.tile([128, 512], F32, tag="pp")
                    for kb in range(KB):
                        nc.tensor.matmul(
                            ps[:, 0:HL * DH],
                            xT_sb[kb][:, nb * 128:(nb + 1) * 128],
                            wv_sb[kb][:],
                            start=(kb == 0),
                            stop=(kb == KB - 1),
                        )
                    nc.vector.tensor_tensor(vhat[nb][:], ps[:, 0:HL * DH], bvb_sb[:], ADD)

            def emit_qk_chunk(h, i):
                """i in 0..7 -> (q|k, column chunk c)."""
                qk, c = divmod(i, 4)
                w_sb, b_sb, dst, sc = [
                    (wq_sb, bq_sb, qT[h], SCALE),
                    (wk_sb, bk_sb, kT[h], 1.0),
                ][qk]
                with nc.named_scope(f"qkproj{h}"):
                    ps = pp.tile([128, 512], F32, tag="pp", name=f"psqk{h}_{i}")
                    for kb in range(KB):
                        nc.tensor.matmul(
                            ps[0:D, :],
                            w_sb[kb][:, h * D:(h + 1) * D],
                            xT_sb[kb][:, c * 512:(c + 1) * 512],
                            start=(kb == 0),
                            stop=(kb == KB - 1),
                        )
                    nc.vector.tensor_scalar(
                        dst[:, c * 512:(c + 1) * 512],
                        ps[0:D, :],
                        sc,
                        b_sb[:, h:h + 1],
                        MULT,
                        ADD,
                    )

            def emit_bpb():
                with nc.named_scope("bpb"):
                    for off, w in [(0, 512), (512, 256)]:
                        ps = pp.tile([128, 512], F32, tag="pp")
                        nc.tensor.matmul(
                            ps[:, 0:w], ones_sb[:], bp_sb[:, off:off + w],
                            start=True, stop=True,
                        )
                        nc.vector.tensor_copy(bpb_sb[:, off:off + w], ps[:, 0:w])

            # persistent normalized qh=1 outputs (fed straight to out-proj)
            otq1 = [
                [
                    cpool.tile([D, 512], BF16, tag=f"otq1_{h}_{j}", name=f"otq1_{h}_{j}")
                    for j in range(2)
                ]
                for h in range(HL)
            ]

            def emit_norm(job, after=(None, None)):
                h, qh, acc_sb, rec = job
                with nc.named_scope(f"norm{h}_{qh}"):
                    for j in range(2):
                        bc = pp.tile([128, 512], F32, tag="pp")
                        nc.tensor.matmul(
                            bc[0:D, :],
                            ones_sb[:, 0:D],
                            rec[:, j * 512:(j + 1) * 512],
                            start=True,
                            stop=True,
                        )
                        if qh == 0:
                            ot = npool.tile([D, 512], BF16, tag="ot")
                        else:
                            ot = otq1[h][j]
                        nc.vector.tensor_tensor(
                            ot[:], acc_sb[0:D, j * 512:(j + 1) * 512], bc[0:D, :], MULT
                        )
                        if qh == 0:
                            # scatter the 4 row-blocks (sync queue: idle by now)
                            for q in range(4):
                                nc.sync.dma_start(
                                    ot_d[j * 4 + q, h * D:(h + 1) * D, :],
                                    ot[:, q * 128:(q + 1) * 128],
                                )
                        if after[j] is not None:
                            after[j]()

            def emit_out(nb):
                with nc.named_scope(f"oproj{nb}"):
                    otn = npool.tile([128, 3, 128], BF16, tag="otn", bufs=4)
                    src = ot_d[nb].rearrange("(g p) i -> p g i", p=128)
                    eng = nc.gpsimd if nb % 2 == 0 else nc.scalar
                    eng.dma_start(otn[:], src)
                    osb = npool.tile([128, E], F32, tag="osb", bufs=3)
                    for off, w in [(0, 512), (512, 256)]:
                        po = pp.tile([128, 512], F32, tag="pp")
                        for g in range(3):
                            nc.tensor.matmul(
                                po[:, 0:w],
                                otn[:, g, :],
                                wp_sb[g][:, off:off + w],
                                start=(g == 0),
                                stop=(g == 2),
                            )
                        nc.vector.tensor_tensor(
                            osb[:, off:off + w], po[:, 0:w], bpb_sb[:, off:off + w], ADD
                        )
                    nc.gpsimd.dma_start(out_d[nb * 128:(nb + 1) * 128, :], osb[:])

            def emit_out_direct(nb):
                """qh=1 out-proj from SBUF: 4 per-head K=96 accumulating
                matmuls per column chunk, no DRAM staging roundtrip."""
                j, i = divmod(nb - 8, 4)
                with nc.named_scope(f"oprojd{nb}"):
                    osb = npool.tile([128, E], F32, tag="osb", bufs=3)
                    for off, w in [(0, 512), (512, 256)]:
                        po = pp.tile([128, 512], F32, tag="pp")
                        for h in range(HL):
                            nc.tensor.matmul(
                                po[:, 0:w],
                                otq1[h][j][:, i * 128:(i + 1) * 128],
                                wph_sb[h][:, off:off + w],
                                start=(h == 0),
                                stop=(h == HL - 1),
                            )
                        nc.vector.tensor_tensor(
                            osb[:, off:off + w], po[:, 0:w], bpb_sb[:, off:off + w], ADD
                        )
                    nc.gpsimd.dma_start(out_d[nb * 128:(nb + 1) * 128, :], osb[:])

            # ---- pre-phase: h0 critical qk chunks, then V projection ----
            # critical = q chunks for qh0 (c0,c1) + all k chunks; q c2/c3 are
            # only needed by the qh=1 pass and are deferred as fill jobs.
            CRIT = [0, 4, 1, 5, 6, 7]   # (q,c0),(k,c0),(q,c1),(k,c1),(k,c2),(k,c3)
            DEFER = [2, 3]              # (q,c2),(q,c3)
            for i in CRIT:
                emit_qk_chunk(0, i)
            for nb in range(12):
                emit_vproj(nb)

            # fill-job queues per pass: (min_kb, closure)
            def J(fn, *a, min_kb=0):
                return (min_kb, lambda: fn(*a))

            fills = {
                (0, 0): [J(emit_vproj, nb) for nb in range(12, NB)]
                        + [J(emit_qk_chunk, 1, i) for i in CRIT],
                (1, 0): [J(emit_qk_chunk, 2, i) for i in CRIT]
                        + [J(emit_qk_chunk, 0, i) for i in DEFER],
                (2, 0): [J(emit_qk_chunk, 3, i) for i in CRIT]
                        + [J(emit_qk_chunk, 1, i) for i in DEFER],
                (3, 0): [J(emit_qk_chunk, 2, i) for i in DEFER]
                        + [J(emit_qk_chunk, 3, i) for i in DEFER]
                        + [J(emit_bpb)],
                # q1 passes: out-proj for the qh=0 half interleaves in.
                # nb0-7 need every head's qh=0 norm; the last one (h3,q0) is
                # emitted at kb==6 of pass (0,1), so gate on kb>=8 there.
                (0, 1): [J(emit_out, nb, min_kb=8) for nb in range(0, 2)],
                (1, 1): [J(emit_out, nb) for nb in range(2, 5)],
                (2, 1): [J(emit_out, nb) for nb in range(5, 8)],
                (3, 1): [],
            }

            pending = None
            for qh in range(2):
                for h in range(HL):
                    jobs = fills[(h, qh)]
                    popped = 0
                    with nc.named_scope(f"attn{h}_{qh}"):
                        acc = pattn.tile([DH, 1024], F32, tag="acc")

                        def emit_pv(kbp, pt):
                            for j in range(2):
                                nc.tensor.matmul(
                                    acc[:, j * 512:(j + 1) * 512],
                                    vhat[kbp][:, h * DH:(h + 1) * DH],
                                    pt[:, j * 512:(j + 1) * 512],
                                    start=(kbp == 0),
                                    stop=(kbp == NB - 1),
                                )

                        prev = None
                        for kb in range(NB):
                            et = pattn.tile([128, 1024], F32, tag="et", bufs=2)
                            for j in range(2):
                                c = 2 * qh + j
                                nc.tensor.matmul(
                                    et[:, j * 512:(j + 1) * 512],
                                    kT[h][:, kb * 128:(kb + 1) * 128],
                                    qT[h][:, c * 512:(c + 1) * 512],
                                    start=True,
                                    stop=True,
                                )
                            # PV runs one step behind so exp(kb) overlaps it
                            if prev is not None:
                                emit_pv(kb - 1, prev)
                            pt = ptpool.tile([128, 1024], BF16, tag="pt")
                            nc.scalar.activation(pt[:], et[:], AF.Exp)
                            prev = pt
                            if kb == 6 and pending is not None:
                                emit_norm(pending)
                                pending = None
                            # paced fill: spread jobs evenly over eligible kbs
                            target = (kb + 1) * len(jobs) // NB
                            while popped < len(jobs) and popped < target \
                                    and jobs[popped][0] <= kb:
                                jobs[popped][1]()
                                popped += 1
                        while popped < len(jobs):
                            jobs[popped][1]()
                            popped += 1
                        emit_pv(NB - 1, prev)
                        # reciprocal chain now (off the next pass / tail
                        # critical path); sums read straight from PSUM
                        sums = npool.tile([1, 1024], F32, tag="sums", bufs=1)
                        nc.vector.tensor_copy(sums[:], acc[D:DH, :])
                        rec32 = npool.tile([1, 1024], F32, tag="rec32", bufs=1)
                        nc.vector.reciprocal_approx_fast(rec32[:], sums[:])
                        rec = npool.tile([1, 1024], F32R, tag="rec")
                        nc.vector.tensor_copy(rec[:], rec32[:])
                        acc_sb = npool.tile([DH, 1024], F32, tag="acc_sb")
                        nc.vector.tensor_copy(acc_sb[:, 0:512], acc[:, 0:512])
                        nc.scalar.copy(acc_sb[:, 512:1024], acc[:, 512:1024])
                        pending = (h, qh, acc_sb, rec)

            # ---- tail: last norm overlapped with SBUF-direct out-proj ----
            emit_norm(
                pending,
                after=(
                    lambda: [emit_out_direct(nb) for nb in range(8, 12)],
                    lambda: [emit_out_direct(nb) for nb in range(12, NB)],
                ),
            )

    nc.compile()
    return nc


def _shard(x, Wqkv, bqkv, Wproj, bproj):
    """Build per-core input maps. Core c -> (batch c//2, head-group c%2)."""
    BF = ml_dtypes.bfloat16
    Wr = np.ascontiguousarray(Wqkv.reshape(E, H, D, 3))
    br = np.ascontiguousarray(bqkv.reshape(H, D, 3))
    ones = np.ones((1, 128), np.float32)
    in_maps = []
    for c in range(NC):
        bb, hg = divmod(c, 2)
        hs = slice(hg * HL, (hg + 1) * HL)
        wq = np.ascontiguousarray(Wr[:, hs, :, 0].reshape(E, EL).astype(BF))
        wk = np.ascontiguousarray(Wr[:, hs, :, 1].reshape(E, EL).astype(BF))
        wv = np.zeros((E, HL, DH), np.float32)
        wv[:, :, :D] = Wr[:, hs, :, 2]
        bq = np.ascontiguousarray((br[hs, :, 0] * SCALE).T)  # [D, HL], pre-scaled
        bk = np.ascontiguousarray(br[hs, :, 1].T)
        bv = np.zeros((HL, DH), np.float32)
        bv[:, :D] = br[hs, :, 2]
        bv[:, D] = 1.0  # denominator ones column
        wp = np.ascontiguousarray(Wproj[hg * EL:(hg + 1) * EL, :].astype(BF))
        wph = np.ascontiguousarray(
            Wproj[hg * EL:(hg + 1) * EL, :].reshape(HL, D, E).astype(BF)
        )
        bp = bproj if hg == 0 else np.zeros_like(bproj)
        in_maps.append({
            "xT": np.ascontiguousarray(x[bb].T.astype(BF)),
            "wq": wq,
            "wk": wk,
            "wv": np.ascontiguousarray(wv.reshape(E, HL * DH).astype(BF)),
            "bq": bq,
            "bk": bk,
            "bv": np.ascontiguousarray(bv.reshape(1, HL * DH)),
            "wp": wp,
            "wph": wph,
            "bp": np.ascontiguousarray(bp.reshape(1, E)),
            "ones": ones,
        })
    return in_maps


def kernel(x, Wqkv, bqkv, Wproj, bproj):
    global _COMPILED, LAST_EXEC_NS, LAST_RESULTS
    x = np.asarray(x, dtype=np.float32)
    Wqkv = np.asarray(Wqkv, dtype=np.float32)
    bqkv = np.asarray(bqkv, dtype=np.float32)
    Wproj = np.asarray(Wproj, dtype=np.float32)
    bproj = np.asarray(bproj, dtype=np.float32)

    if _COMPILED is None:
        _COMPILED = _build()
    nc = _COMPILED

    in_maps = _shard(x, Wqkv, bqkv, Wproj, bproj)
    trace = bool(int(os.environ.get("BASS_MHA_TRACE", "0")))
    try:
        res = run_bass_kernel_spmd(nc, in_maps, list(range(NC)), trace=trace)
    except Exception:
        _device_reset()
        res = run_bass_kernel_spmd(nc, in_maps, list(range(NC)), trace=trace)
    LAST_EXEC_NS = res.exec_time_ns
    LAST_RESULTS = res

    out = np.empty((B, N, E), np.float32)
    for bb in range(B):
        out[bb] = res.results[2 * bb]["out"] + res.results[2 * bb + 1]["out"]
    return out
